# revision 20
# baseline (speedup 1.0000x reference)
"""Trainium2 Bass kernel for GQA attention (dense transformer block).

Model: B=4, S=2048, D=2048, 16 q-heads / 4 kv-heads, head_dim=128, RoPE,
non-causal SDPA, output projection.

Sharding (8 cores): 4-way data-parallel over batch x 2-way tensor-parallel
over kv-head pairs. Core c handles batch c//2 and kv heads {2r, 2r+1}
(q heads 8r..8r+7) where r = c%2. Each core emits a partial (S, D) output
(its 8 heads through its wo row-slice); the host sums the two TP partials
per batch.

On-chip layout strategy: the host passes x pre-transposed (xT: [D, S]) and
weights pre-sliced, with wq/wk columns de-interleaved per head (rotate-half
RoPE layout). Every matmul in the chain then takes its operands in natural
layout with zero on-chip transposes:

  QT[qcol, s]   = wq_sh.T @ x     (lhsT=wq_sh, rhs=xT)
  KT[kcol, s]   = wk_sh.T @ x
  V[s, vcol]    = x @ wv_sh       (lhsT=xT, rhs=wv_sh)
  RoPE on QT/KT: partition-half swap + elementwise (DVE)
  scoresT[k, q] = KT_h.T-slice @ QT_h   (lhsT=KT_h[:,ktile], rhs=QT_h[:,qtile])
  PT[k, q]      = exp(scoresT * scale)  (ACT, fused scale; no max-sub needed:
                                         |scores*scale| < ~8 for this input dist)
  OT[hd, q]     = sum_k V_h[ktile].T @ PT[ktile]      (accumulated in PSUM)
  den[*, q]     = sum_k ones.T @ PT[ktile]            (softmax denominator,
                                                       broadcast to 128 rows)
  OTn           = OT * recip(den)                     (DVE, fused with PSUM->SBUF)
  out[s, e]     = sum_h OTn_h.T-slice @ wo_h          (accumulated in PSUM)

All matmul operands are bf16 (PE runs 1 cycle/row for bf16 vs 4 for fp32);
accumulation and softmax are fp32 in PSUM.
"""
import sys
for _p in ("/opt/trn_rl_repo",):
    if _p not in sys.path:
        sys.path.insert(0, _p)

import numpy as np
import ml_dtypes
from contextlib import ExitStack

import concourse.bass as bass
import concourse.tile as tile
from concourse import mybir
from concourse.bass_utils import run_bass_kernel_spmd

BF16 = mybir.dt.bfloat16
F32 = mybir.dt.float32
AF = mybir.ActivationFunctionType

# Model dims (hardcoded per problem spec)
B, S, D = 4, 2048, 2048
NH, NKV, HD = 16, 4, 128
NCORES = 8
HPC = 8          # q heads per core
KVPC = 2         # kv heads per core
QCOLS = HPC * HD     # 1024
KVCOLS = KVPC * HD   # 256
SCALE = 1.0 / float(np.sqrt(HD))

DT = D // 128    # 16 contraction tiles
ST = S // 128    # 16 token tiles of 128
SQ = S // 512    # 4 token tiles of 512
KT = S // 128    # 16 key tiles of 128
ET = D // 512    # 4 output-embed tiles of 512

_NC_CACHE = None


def _rope_apply(nc, pool, ps, dst, cos_ap, sin_ap):
    """Rotate-half RoPE on a [128, 512] PSUM tile -> bf16 SBUF dst slice.

    ps rows 0:64 = first-half pair elements, 64:128 = second-half.
    dst = ps * cos + swap_halves(ps) * sin_signed  (sin rows 0:64 negated
    host-side)."""
    tcos = pool.tile([128, 512], BF16, tag="tcos")
    nc.vector.tensor_mul(tcos[:], ps[:], cos_ap)
    rot = pool.tile([128, 512], BF16, tag="rot")
    nc.vector.tensor_copy(rot[0:64, :], ps[64:128, :])
    nc.vector.tensor_copy(rot[64:128, :], ps[0:64, :])
    tsin = pool.tile([128, 512], BF16, tag="tsin")
    nc.vector.tensor_mul(tsin[:], rot[:], sin_ap)
    nc.vector.tensor_add(dst, tcos[:], tsin[:])


def build_kernel(repeat=1, sc_bufs=2, ot_bufs=2, den_bufs=2, c_bufs=2, lookahead=2, pt_bufs=4, pa_bufs=4,
                 timing_mode=False):
    nc = bass.Bass()
    if timing_mode:
        # Timing variant: big tensors are internal (uninitialized) DRAM so a
        # call ships ~0 bytes over the PJRT tunnel; dummy/probe keep the
        # ExternalInput/Output contract alive. Compute is identical.
        kin = {"kind": "Internal"}
        kout = {"kind": "Internal"}
    else:
        kin = {"kind": "ExternalInput"}
        kout = {"kind": "ExternalOutput"}
    xT = nc.dram_tensor("xT", [D, S], BF16, **kin)
    wq = nc.dram_tensor("wq", [D, QCOLS], BF16, **kin)
    wk = nc.dram_tensor("wk", [D, KVCOLS], BF16, **kin)
    wv = nc.dram_tensor("wv", [D, KVCOLS], BF16, **kin)
    wo = nc.dram_tensor("wo", [QCOLS, D], BF16, **kin)
    cosT = nc.dram_tensor("cosT", [HD, S], BF16, **kin)
    sinT = nc.dram_tensor("sinT", [HD, S], BF16, **kin)
    out = nc.dram_tensor("out", [S, D], F32, **kout)
    if timing_mode:
        dummy = nc.dram_tensor("t_dummy", [128, 128], F32, kind="ExternalInput")
        probe = nc.dram_tensor("t_probe", [128, 128], F32, kind="ExternalOutput")

    with tile.TileContext(nc) as tc, ExitStack() as ctx:
        persist = ctx.enter_context(tc.tile_pool(name="persist", bufs=1))

        qt_rot = [persist.tile([HD, S], BF16, name=f"qt{h}", tag=f"qt{h}") for h in range(HPC)]
        kt_rot = [persist.tile([HD, S], BF16, name=f"kt{g}", tag=f"kt{g}") for g in range(KVPC)]
        v_sb = [persist.tile([128, KVCOLS], BF16, name=f"v{i}", tag=f"v{i}") for i in range(ST)]
        ones_sb = persist.tile([128, 128], BF16, name="ones", tag="ones")
        nc.any.memset(ones_sb[:], 1.0)

        # ---------------- Phase A: projections + RoPE ----------------
        with (
            tc.tile_pool(name="pa_in", bufs=1) as pa_in,
            tc.tile_pool(name="rope_tmp", bufs=3) as rope_tmp,
            tc.tile_pool(name="pa_ps", bufs=pa_bufs, space="PSUM") as pa_ps,
        ):
            cos_sb = pa_in.tile([HD, S], BF16, name="cos", tag="cos")
            sin_sb = pa_in.tile([HD, S], BF16, name="sin", tag="sin")
            nc.sync.dma_start(cos_sb[:], cosT[:])
            nc.sync.dma_start(sin_sb[:], sinT[:])

            # xT split into 512-column quarters so the first projection
            # matmuls aren't gated on full 512KB tiles; wq deferred (the Q
            # projection runs last in phase A).
            xt_sb = [[pa_in.tile([128, 512], BF16, name=f"xt{d}_{n}", tag=f"xt{d}_{n}")
                      for n in range(SQ)] for d in range(DT)]
            wq_sb = [pa_in.tile([128, QCOLS], BF16, name=f"wq{d}", tag=f"wq{d}") for d in range(DT)]
            wk_sb = [pa_in.tile([128, KVCOLS], BF16, name=f"wk{d}", tag=f"wk{d}") for d in range(DT)]
            wv_sb = [pa_in.tile([128, KVCOLS], BF16, name=f"wv{d}", tag=f"wv{d}") for d in range(DT)]
            # DMA order follows first-use order (wk -> xT n-major -> wv -> wq).
            for d in range(DT):
                nc.sync.dma_start(wk_sb[d][:], wk[d * 128:(d + 1) * 128, :])
            for n in range(SQ):
                for d in range(DT):
                    nc.sync.dma_start(xt_sb[d][n][:],
                                      xT[d * 128:(d + 1) * 128, n * 512:(n + 1) * 512])
            for d in range(DT):
                nc.sync.dma_start(wv_sb[d][:], wv[d * 128:(d + 1) * 128, :])
            for d in range(DT):
                nc.sync.dma_start(wq_sb[d][:], wq[d * 128:(d + 1) * 128, :])

            # KT projection + RoPE (needed first by every attention head)
            for _repa in range(repeat):
             for g in range(KVPC):
                for n in range(SQ):
                    ps = pa_ps.tile([128, 512], F32, tag="proj")
                    for d in range(DT):
                        nc.tensor.matmul(
                            ps[:],
                            wk_sb[d][:, g * 128:(g + 1) * 128],
                            xt_sb[d][n][:],
                            start=(d == 0), stop=(d == DT - 1),
                        )
                    nsl = slice(n * 512, (n + 1) * 512)
                    _rope_apply(nc, rope_tmp, ps, kt_rot[g][:, nsl],
                                cos_sb[:, nsl], sin_sb[:, nsl])

             # V projection (natural [s, vcol] layout; no RoPE)
             for i in range(ST):
                ps = pa_ps.tile([128, KVCOLS], F32, tag="proj")
                for d in range(DT):
                    nc.tensor.matmul(
                        ps[:],
                        xt_sb[d][i // 4][:, (i % 4) * 128:(i % 4 + 1) * 128],
                        wv_sb[d][:],
                        start=(d == 0), stop=(d == DT - 1),
                    )
                nc.scalar.copy(v_sb[i][:], ps[:])

             # QT projection + RoPE (n-outer so phase B's first q-tile
             # has every head ready as early as possible)
             for n in range(SQ):
                for h in range(HPC):
                    ps = pa_ps.tile([128, 512], F32, tag="proj")
                    for d in range(DT):
                        nc.tensor.matmul(
                            ps[:],
                            wq_sb[d][:, h * 128:(h + 1) * 128],
                            xt_sb[d][n][:],
                            start=(d == 0), stop=(d == DT - 1),
                        )
                    nsl = slice(n * 512, (n + 1) * 512)
                    _rope_apply(nc, rope_tmp, ps, qt_rot[h][:, nsl],
                                cos_sb[:, nsl], sin_sb[:, nsl])

        # ---------------- Phases B + C ----------------
        with (
            tc.tile_pool(name="wo_sb", bufs=1) as wo_pool,
            tc.tile_pool(name="pt", bufs=pt_bufs) as pt_pool,
            tc.tile_pool(name="rb", bufs=2) as rb_pool,
            tc.tile_pool(name="otn", bufs=2) as otn_pool,
            tc.tile_pool(name="osb", bufs=3) as out_pool,
            tc.tile_pool(name="pb_sc", bufs=sc_bufs, space="PSUM") as ps_sc,
            tc.tile_pool(name="pb_ot", bufs=ot_bufs, space="PSUM") as ps_ot,
            tc.tile_pool(name="pb_den", bufs=den_bufs, space="PSUM") as ps_den,
            tc.tile_pool(name="pc_ps", bufs=c_bufs, space="PSUM") as ps_c,
        ):
            wo_sb = [wo_pool.tile([128, D], BF16, name=f"wo{h}", tag=f"wo{h}") for h in range(HPC)]
            for h in range(HPC):
                nc.sync.dma_start(wo_sb[h][:], wo[h * 128:(h + 1) * 128, :])

            LOOKAHEAD = lookahead  # scores matmuls emitted ahead of dependent OT/den
            for _rep in range(repeat):
             for qt in range(SQ):
                qsl = slice(qt * 512, (qt + 1) * 512)
                otn_tiles = []
                # Phase B: attention for all heads at this q-tile.
                # Software-pipelined: scores mm for k+LOOKAHEAD is emitted
                # before OT/den mms for k, so the PE has independent work
                # while ACT computes exp(k).
                for h in range(HPC):
                    g = h // 4
                    gsl = slice(g * 128, (g + 1) * 128)
                    ot_ps = ps_ot.tile([HD, 512], F32, tag="ot")
                    den_ps = ps_den.tile([128, 512], F32, tag="den")
                    pts = [None] * KT

                    def emit_scores(k):
                        sc_ps = ps_sc.tile([128, 512], F32, tag="sc")
                        nc.tensor.matmul(
                            sc_ps[:],
                            kt_rot[g][:, k * 128:(k + 1) * 128],
                            qt_rot[h][:, qsl],
                            start=True, stop=True,
                        )
                        pt = pt_pool.tile([128, 512], BF16, tag="pt")
                        nc.scalar.activation(pt[:], sc_ps[:], AF.Exp, scale=SCALE)
                        pts[k] = pt

                    for k in range(LOOKAHEAD):
                        emit_scores(k)
                    for k in range(KT):
                        if k + LOOKAHEAD < KT:
                            emit_scores(k + LOOKAHEAD)
                        pt = pts[k]
                        nc.tensor.matmul(
                            ot_ps[:], v_sb[k][:, gsl], pt[:],
                            start=(k == 0), stop=(k == KT - 1),
                        )
                        nc.tensor.matmul(
                            den_ps[:], ones_sb[:], pt[:],
                            start=(k == 0), stop=(k == KT - 1),
                        )
                        pts[k] = None
                    rb = rb_pool.tile([128, 512], F32, tag="rb")
                    nc.vector.reciprocal(rb[:], den_ps[:])
                    otn = otn_pool.tile([HD, 512], BF16, name=f"otn{h}", tag=f"otn{h}")
                    nc.vector.tensor_mul(otn[:], ot_ps[:], rb[:])
                    otn_tiles.append(otn)

                # Phase C: output projection for this q-tile's tokens
                for s4 in range(4):
                    st = qt * 4 + s4
                    ssl = slice(s4 * 128, (s4 + 1) * 128)
                    osb = out_pool.tile([128, D], F32, tag="osb")
                    for et in range(ET):
                        o_ps = ps_c.tile([128, 512], F32, tag="oc")
                        for h in range(HPC):
                            nc.tensor.matmul(
                                o_ps[:],
                                otn_tiles[h][:, ssl],
                                wo_sb[h][:, et * 512:(et + 1) * 512],
                                start=(h == 0), stop=(h == HPC - 1),
                            )
                        nc.scalar.copy(osb[:, et * 512:(et + 1) * 512], o_ps[:])
                    nc.sync.dma_start(out[st * 128:(st + 1) * 128, :], osb[:])
                    if timing_mode and _rep == repeat - 1 and st == ST - 1:
                        # tiny externally visible result so the host can
                        # block on kernel completion
                        pad = out_pool.tile([128, 128], F32, name="pad", tag="pad")
                        nc.sync.dma_start(pad[:], dummy[:])
                        nc.vector.tensor_add(pad[:], pad[:], osb[:, 0:128])
                        nc.sync.dma_start(probe[:], pad[:])

    return nc


def _prep_inputs(x, freqs_cos, freqs_sin, wq, wk, wv, wo):
    bf16 = ml_dtypes.bfloat16
    f32 = np.float32
    x = np.asarray(x, f32)
    freqs_cos = np.asarray(freqs_cos, f32)
    freqs_sin = np.asarray(freqs_sin, f32)
    wq = np.asarray(wq, f32)
    wk = np.asarray(wk, f32)
    wv = np.asarray(wv, f32)
    wo = np.asarray(wo, f32)

    # cos/sin transposed + duplicated for the two rotate-half blocks;
    # sin first half negated (sign folded into the table).
    cosT = np.concatenate([freqs_cos.T, freqs_cos.T], axis=0).astype(bf16)
    sinT = np.concatenate([-freqs_sin.T, freqs_sin.T], axis=0).astype(bf16)
    cosT = np.ascontiguousarray(cosT)
    sinT = np.ascontiguousarray(sinT)

    # De-interleave RoPE pairs within each head: [0,2,...,126, 1,3,...,127]
    perm = np.concatenate([np.arange(0, HD, 2), np.arange(1, HD, 2)])
    qp = (np.arange(NH)[:, None] * HD + perm[None, :]).reshape(-1)
    kp = (np.arange(NKV)[:, None] * HD + perm[None, :]).reshape(-1)
    wq_p = wq[:, qp]
    wk_p = wk[:, kp]

    in_maps = []
    for c in range(NCORES):
        b, r = c // 2, c % 2
        in_maps.append({
            "xT": np.ascontiguousarray(x[b].T).astype(bf16),
            "wq": np.ascontiguousarray(wq_p[:, r * QCOLS:(r + 1) * QCOLS]).astype(bf16),
            "wk": np.ascontiguousarray(wk_p[:, r * KVCOLS:(r + 1) * KVCOLS]).astype(bf16),
            "wv": np.ascontiguousarray(wv[:, r * KVCOLS:(r + 1) * KVCOLS]).astype(bf16),
            "wo": np.ascontiguousarray(wo[r * QCOLS:(r + 1) * QCOLS, :]).astype(bf16),
            "cosT": cosT,
            "sinT": sinT,
        })
    return in_maps


def _legalize_waits(nc):
    """Hoist extra sync-waits onto single-wait NoOps: this walrus build
    accepts only one sync-wait command per instruction."""
    n = 0
    for func in nc.m.functions:
        for bb in func.blocks:
            insts = list(bb.instructions)
            out = []
            changed = False
            for inst in insts:
                si = inst.sync_info
                waits = list(si.on_wait) if si and si.on_wait else []
                if len(waits) > 1:
                    for w in waits[:-1]:
                        nop = mybir.InstNoOp(name=f"I-waitsplit-{n}", ins=[], outs=[])
                        n += 1
                        nop.engine = inst.engine
                        nop.sync_info = mybir.SyncInfo(on_wait=[w], on_update=[])
                        out.append(nop)
                    si.on_wait = [waits[-1]]
                    changed = True
                out.append(inst)
            if changed:
                bb.instructions = out
    return n


TUNED = dict(sc_bufs=3, den_bufs=1)


def get_nc():
    global _NC_CACHE
    if _NC_CACHE is None:
        nc = build_kernel(**TUNED)
        _legalize_waits(nc)
        _NC_CACHE = nc
    return _NC_CACHE


def run(in_maps, **kwargs):
    return run_bass_kernel_spmd(get_nc(), in_maps, list(range(NCORES)), **kwargs)


def kernel(x, freqs_cos, freqs_sin, wq, wk, wv, wo):
    in_maps = _prep_inputs(x, freqs_cos, freqs_sin, wq, wk, wv, wo)
    res = run(in_maps)
    parts = [res.results[c]["out"] for c in range(NCORES)]
    out = np.stack([parts[2 * b] + parts[2 * b + 1] for b in range(B)], axis=0)
    return out.astype(np.float32)


# revision 21
# speedup vs baseline: 9.6996x; 9.6996x over previous
"""Trainium2 Bass kernel for GQA attention (dense transformer block).

Model: B=4, S=2048, D=2048, 16 q-heads / 4 kv-heads, head_dim=128, RoPE,
non-causal SDPA, output projection.

Sharding (8 cores): 4-way data-parallel over batch x 2-way tensor-parallel
over kv-head pairs. Core c handles batch c//2 and kv heads {2r, 2r+1}
(q heads 8r..8r+7) where r = c%2. Each core emits a partial (S, D) output
(its 8 heads through its wo row-slice); the host sums the two TP partials
per batch.

On-chip layout strategy: the host passes x pre-transposed (xT: [D, S]) and
weights pre-sliced, with wq/wk columns de-interleaved per head (rotate-half
RoPE layout). Every matmul in the chain then takes its operands in natural
layout with zero on-chip transposes:

  QT[qcol, s]   = wq_sh.T @ x     (lhsT=wq_sh, rhs=xT)
  KT[kcol, s]   = wk_sh.T @ x
  V[s, vcol]    = x @ wv_sh       (lhsT=xT, rhs=wv_sh)
  RoPE on QT/KT: partition-half swap + elementwise (DVE)
  scoresT[k, q] = KT_h.T-slice @ QT_h   (lhsT=KT_h[:,ktile], rhs=QT_h[:,qtile])
  PT[k, q]      = exp(scoresT * scale)  (ACT, fused scale; no max-sub needed:
                                         |scores*scale| < ~8 for this input dist)
  OT[hd, q]     = sum_k V_h[ktile].T @ PT[ktile]      (accumulated in PSUM)
  den[*, q]     = sum_k ones.T @ PT[ktile]            (softmax denominator,
                                                       broadcast to 128 rows)
  OTn           = OT * recip(den)                     (DVE, fused with PSUM->SBUF)
  out[s, e]     = sum_h OTn_h.T-slice @ wo_h          (accumulated in PSUM)

All matmul operands are bf16 (PE runs 1 cycle/row for bf16 vs 4 for fp32);
accumulation and softmax are fp32 in PSUM.
"""
import sys
for _p in ("/opt/trn_rl_repo",):
    if _p not in sys.path:
        sys.path.insert(0, _p)

import numpy as np
import ml_dtypes
from contextlib import ExitStack

import concourse.bass as bass
import concourse.tile as tile
from concourse import mybir
from concourse.bass_utils import run_bass_kernel_spmd

BF16 = mybir.dt.bfloat16
F32 = mybir.dt.float32
AF = mybir.ActivationFunctionType

# Model dims (hardcoded per problem spec)
B, S, D = 4, 2048, 2048
NH, NKV, HD = 16, 4, 128
NCORES = 8
HPC = 8          # q heads per core
KVPC = 2         # kv heads per core
QCOLS = HPC * HD     # 1024
KVCOLS = KVPC * HD   # 256
SCALE = 1.0 / float(np.sqrt(HD))

DT = D // 128    # 16 contraction tiles
ST = S // 128    # 16 token tiles of 128
SQ = S // 512    # 4 token tiles of 512
KT = S // 128    # 16 key tiles of 128
ET = D // 512    # 4 output-embed tiles of 512

_NC_CACHE = None


def _rope_apply(nc, pool, ps, dst, cos_ap, sin_ap):
    """Rotate-half RoPE on a [128, 512] PSUM tile -> bf16 SBUF dst slice.

    ps rows 0:64 = first-half pair elements, 64:128 = second-half.
    dst = ps * cos + swap_halves(ps) * sin_signed  (sin rows 0:64 negated
    host-side)."""
    tcos = pool.tile([128, 512], BF16, tag="tcos")
    nc.vector.tensor_mul(tcos[:], ps[:], cos_ap)
    rot = pool.tile([128, 512], BF16, tag="rot")
    nc.vector.tensor_copy(rot[0:64, :], ps[64:128, :])
    nc.vector.tensor_copy(rot[64:128, :], ps[0:64, :])
    tsin = pool.tile([128, 512], BF16, tag="tsin")
    nc.vector.tensor_mul(tsin[:], rot[:], sin_ap)
    nc.vector.tensor_add(dst, tcos[:], tsin[:])


def build_kernel(repeat=1, sc_bufs=2, ot_bufs=2, den_bufs=2, c_bufs=2, lookahead=2, pt_bufs=4, pa_bufs=4,
                 timing_mode=False):
    nc = bass.Bass()
    if timing_mode:
        # Timing variant: big tensors are internal (uninitialized) DRAM so a
        # call ships ~0 bytes over the PJRT tunnel; dummy/probe keep the
        # ExternalInput/Output contract alive. Compute is identical.
        kin = {"kind": "Internal"}
        kout = {"kind": "Internal"}
    else:
        kin = {"kind": "ExternalInput"}
        kout = {"kind": "ExternalOutput"}
    xT = nc.dram_tensor("xT", [D, S], BF16, **kin)
    wq = nc.dram_tensor("wq", [D, QCOLS], BF16, **kin)
    wk = nc.dram_tensor("wk", [D, KVCOLS], BF16, **kin)
    wv = nc.dram_tensor("wv", [D, KVCOLS], BF16, **kin)
    wo = nc.dram_tensor("wo", [QCOLS, D], BF16, **kin)
    cosT = nc.dram_tensor("cosT", [HD, S], BF16, **kin)
    sinT = nc.dram_tensor("sinT", [HD, S], BF16, **kin)
    out = nc.dram_tensor("out", [S, D], F32, **kout)
    if timing_mode:
        dummy = nc.dram_tensor("t_dummy", [128, 128], F32, kind="ExternalInput")
        probe = nc.dram_tensor("t_probe", [128, 128], F32, kind="ExternalOutput")

    with tile.TileContext(nc) as tc, ExitStack() as ctx:
        persist = ctx.enter_context(tc.tile_pool(name="persist", bufs=1))

        qt_rot = [persist.tile([HD, S], BF16, name=f"qt{h}", tag=f"qt{h}") for h in range(HPC)]
        kt_rot = [persist.tile([HD, S], BF16, name=f"kt{g}", tag=f"kt{g}") for g in range(KVPC)]
        v_sb = [persist.tile([128, KVCOLS], BF16, name=f"v{i}", tag=f"v{i}") for i in range(ST)]
        ones_sb = persist.tile([128, 128], BF16, name="ones", tag="ones")
        nc.any.memset(ones_sb[:], 1.0)

        # ---------------- Phase A: projections + RoPE ----------------
        with (
            tc.tile_pool(name="pa_in", bufs=1) as pa_in,
            tc.tile_pool(name="rope_tmp", bufs=3) as rope_tmp,
            tc.tile_pool(name="pa_ps", bufs=pa_bufs, space="PSUM") as pa_ps,
        ):
            cos_sb = pa_in.tile([HD, S], BF16, name="cos", tag="cos")
            sin_sb = pa_in.tile([HD, S], BF16, name="sin", tag="sin")
            nc.sync.dma_start(cos_sb[:], cosT[:])
            nc.sync.dma_start(sin_sb[:], sinT[:])

            # xT split into 512-column quarters so the first projection
            # matmuls aren't gated on full 512KB tiles; wq deferred (the Q
            # projection runs last in phase A).
            xt_sb = [[pa_in.tile([128, 512], BF16, name=f"xt{d}_{n}", tag=f"xt{d}_{n}")
                      for n in range(SQ)] for d in range(DT)]
            wq_sb = [pa_in.tile([128, QCOLS], BF16, name=f"wq{d}", tag=f"wq{d}") for d in range(DT)]
            wk_sb = [pa_in.tile([128, KVCOLS], BF16, name=f"wk{d}", tag=f"wk{d}") for d in range(DT)]
            wv_sb = [pa_in.tile([128, KVCOLS], BF16, name=f"wv{d}", tag=f"wv{d}") for d in range(DT)]
            # DMA order follows first-use order (wk -> xT n-major -> wv -> wq).
            for d in range(DT):
                nc.sync.dma_start(wk_sb[d][:], wk[d * 128:(d + 1) * 128, :])
            for n in range(SQ):
                for d in range(DT):
                    nc.sync.dma_start(xt_sb[d][n][:],
                                      xT[d * 128:(d + 1) * 128, n * 512:(n + 1) * 512])
            for d in range(DT):
                nc.sync.dma_start(wv_sb[d][:], wv[d * 128:(d + 1) * 128, :])
            for d in range(DT):
                nc.sync.dma_start(wq_sb[d][:], wq[d * 128:(d + 1) * 128, :])

            # KT projection + RoPE (needed first by every attention head)
            for _repa in range(repeat):
             for g in range(KVPC):
                for n in range(SQ):
                    ps = pa_ps.tile([128, 512], F32, tag="proj")
                    for d in range(DT):
                        nc.tensor.matmul(
                            ps[:],
                            wk_sb[d][:, g * 128:(g + 1) * 128],
                            xt_sb[d][n][:],
                            start=(d == 0), stop=(d == DT - 1),
                        )
                    nsl = slice(n * 512, (n + 1) * 512)
                    _rope_apply(nc, rope_tmp, ps, kt_rot[g][:, nsl],
                                cos_sb[:, nsl], sin_sb[:, nsl])

             # V projection (natural [s, vcol] layout; no RoPE)
             for i in range(ST):
                ps = pa_ps.tile([128, KVCOLS], F32, tag="proj")
                for d in range(DT):
                    nc.tensor.matmul(
                        ps[:],
                        xt_sb[d][i // 4][:, (i % 4) * 128:(i % 4 + 1) * 128],
                        wv_sb[d][:],
                        start=(d == 0), stop=(d == DT - 1),
                    )
                nc.scalar.copy(v_sb[i][:], ps[:])

             # QT projection + RoPE (n-outer so phase B's first q-tile
             # has every head ready as early as possible)
             for n in range(SQ):
                for h in range(HPC):
                    ps = pa_ps.tile([128, 512], F32, tag="proj")
                    for d in range(DT):
                        nc.tensor.matmul(
                            ps[:],
                            wq_sb[d][:, h * 128:(h + 1) * 128],
                            xt_sb[d][n][:],
                            start=(d == 0), stop=(d == DT - 1),
                        )
                    nsl = slice(n * 512, (n + 1) * 512)
                    _rope_apply(nc, rope_tmp, ps, qt_rot[h][:, nsl],
                                cos_sb[:, nsl], sin_sb[:, nsl])

        # ---------------- Phases B + C ----------------
        with (
            tc.tile_pool(name="wo_sb", bufs=1) as wo_pool,
            tc.tile_pool(name="pt", bufs=pt_bufs) as pt_pool,
            tc.tile_pool(name="rb", bufs=2) as rb_pool,
            tc.tile_pool(name="otn", bufs=2) as otn_pool,
            tc.tile_pool(name="osb", bufs=3) as out_pool,
            tc.tile_pool(name="pb_sc", bufs=sc_bufs, space="PSUM") as ps_sc,
            tc.tile_pool(name="pb_ot", bufs=ot_bufs, space="PSUM") as ps_ot,
            tc.tile_pool(name="pb_den", bufs=den_bufs, space="PSUM") as ps_den,
            tc.tile_pool(name="pc_ps", bufs=c_bufs, space="PSUM") as ps_c,
        ):
            wo_sb = [wo_pool.tile([128, D], BF16, name=f"wo{h}", tag=f"wo{h}") for h in range(HPC)]
            for h in range(HPC):
                nc.sync.dma_start(wo_sb[h][:], wo[h * 128:(h + 1) * 128, :])

            LOOKAHEAD = lookahead  # scores matmuls emitted ahead of dependent OT/den
            for _rep in range(repeat):
             for qt in range(SQ):
                qsl = slice(qt * 512, (qt + 1) * 512)
                otn_tiles = []
                # Phase B: attention for all heads at this q-tile.
                # Software-pipelined: scores mm for k+LOOKAHEAD is emitted
                # before OT/den mms for k, so the PE has independent work
                # while ACT computes exp(k).
                for h in range(HPC):
                    g = h // 4
                    gsl = slice(g * 128, (g + 1) * 128)
                    ot_ps = ps_ot.tile([HD, 512], F32, tag="ot")
                    den_ps = ps_den.tile([128, 512], F32, tag="den")
                    pts = [None] * KT

                    def emit_scores(k):
                        sc_ps = ps_sc.tile([128, 512], F32, tag="sc")
                        nc.tensor.matmul(
                            sc_ps[:],
                            kt_rot[g][:, k * 128:(k + 1) * 128],
                            qt_rot[h][:, qsl],
                            start=True, stop=True,
                        )
                        pt = pt_pool.tile([128, 512], BF16, tag="pt")
                        nc.scalar.activation(pt[:], sc_ps[:], AF.Exp, scale=SCALE)
                        pts[k] = pt

                    for k in range(LOOKAHEAD):
                        emit_scores(k)
                    for k in range(KT):
                        if k + LOOKAHEAD < KT:
                            emit_scores(k + LOOKAHEAD)
                        pt = pts[k]
                        nc.tensor.matmul(
                            ot_ps[:], v_sb[k][:, gsl], pt[:],
                            start=(k == 0), stop=(k == KT - 1),
                        )
                        nc.tensor.matmul(
                            den_ps[:], ones_sb[:], pt[:],
                            start=(k == 0), stop=(k == KT - 1),
                        )
                        pts[k] = None
                    rb = rb_pool.tile([128, 512], F32, tag="rb")
                    nc.vector.reciprocal(rb[:], den_ps[:])
                    otn = otn_pool.tile([HD, 512], BF16, name=f"otn{h}", tag=f"otn{h}")
                    nc.vector.tensor_mul(otn[:], ot_ps[:], rb[:])
                    otn_tiles.append(otn)

                # Phase C: output projection for this q-tile's tokens
                for s4 in range(4):
                    st = qt * 4 + s4
                    ssl = slice(s4 * 128, (s4 + 1) * 128)
                    osb = out_pool.tile([128, D], F32, tag="osb")
                    for et in range(ET):
                        o_ps = ps_c.tile([128, 512], F32, tag="oc")
                        for h in range(HPC):
                            nc.tensor.matmul(
                                o_ps[:],
                                otn_tiles[h][:, ssl],
                                wo_sb[h][:, et * 512:(et + 1) * 512],
                                start=(h == 0), stop=(h == HPC - 1),
                            )
                        nc.scalar.copy(osb[:, et * 512:(et + 1) * 512], o_ps[:])
                    nc.sync.dma_start(out[st * 128:(st + 1) * 128, :], osb[:])
                    if timing_mode and _rep == repeat - 1 and st == ST - 1:
                        # tiny externally visible result so the host can
                        # block on kernel completion
                        pad = out_pool.tile([128, 128], F32, name="pad", tag="pad")
                        nc.sync.dma_start(pad[:], dummy[:])
                        nc.vector.tensor_add(pad[:], pad[:], osb[:, 0:128])
                        nc.sync.dma_start(probe[:], pad[:])

    return nc


def _prep_inputs(x, freqs_cos, freqs_sin, wq, wk, wv, wo):
    bf16 = ml_dtypes.bfloat16
    f32 = np.float32
    x = np.asarray(x, f32)
    freqs_cos = np.asarray(freqs_cos, f32)
    freqs_sin = np.asarray(freqs_sin, f32)
    wq = np.asarray(wq, f32)
    wk = np.asarray(wk, f32)
    wv = np.asarray(wv, f32)
    wo = np.asarray(wo, f32)

    # cos/sin transposed + duplicated for the two rotate-half blocks;
    # sin first half negated (sign folded into the table).
    cosT = np.concatenate([freqs_cos.T, freqs_cos.T], axis=0).astype(bf16)
    sinT = np.concatenate([-freqs_sin.T, freqs_sin.T], axis=0).astype(bf16)
    cosT = np.ascontiguousarray(cosT)
    sinT = np.ascontiguousarray(sinT)

    # De-interleave RoPE pairs within each head: [0,2,...,126, 1,3,...,127]
    perm = np.concatenate([np.arange(0, HD, 2), np.arange(1, HD, 2)])
    qp = (np.arange(NH)[:, None] * HD + perm[None, :]).reshape(-1)
    kp = (np.arange(NKV)[:, None] * HD + perm[None, :]).reshape(-1)
    wq_p = wq[:, qp]
    wk_p = wk[:, kp]

    in_maps = []
    for c in range(NCORES):
        b, r = c // 2, c % 2
        in_maps.append({
            "xT": np.ascontiguousarray(x[b].T).astype(bf16),
            "wq": np.ascontiguousarray(wq_p[:, r * QCOLS:(r + 1) * QCOLS]).astype(bf16),
            "wk": np.ascontiguousarray(wk_p[:, r * KVCOLS:(r + 1) * KVCOLS]).astype(bf16),
            "wv": np.ascontiguousarray(wv[:, r * KVCOLS:(r + 1) * KVCOLS]).astype(bf16),
            "wo": np.ascontiguousarray(wo[r * QCOLS:(r + 1) * QCOLS, :]).astype(bf16),
            "cosT": cosT,
            "sinT": sinT,
        })
    return in_maps


def _legalize_waits(nc):
    """Hoist extra sync-waits onto single-wait NoOps: this walrus build
    accepts only one sync-wait command per instruction."""
    n = 0
    for func in nc.m.functions:
        for bb in func.blocks:
            insts = list(bb.instructions)
            out = []
            changed = False
            for inst in insts:
                si = inst.sync_info
                waits = list(si.on_wait) if si and si.on_wait else []
                if len(waits) > 1:
                    for w in waits[:-1]:
                        nop = mybir.InstNoOp(name=f"I-waitsplit-{n}", ins=[], outs=[])
                        n += 1
                        nop.engine = inst.engine
                        nop.sync_info = mybir.SyncInfo(on_wait=[w], on_update=[])
                        out.append(nop)
                    si.on_wait = [waits[-1]]
                    changed = True
                out.append(inst)
            if changed:
                bb.instructions = out
    return n


TUNED = dict(sc_bufs=3, den_bufs=1)


def get_nc():
    global _NC_CACHE
    if _NC_CACHE is None:
        nc = build_kernel(**TUNED)
        _legalize_waits(nc)
        _NC_CACHE = nc
    return _NC_CACHE


def run(in_maps, **kwargs):
    return run_bass_kernel_spmd(get_nc(), in_maps, list(range(NCORES)), **kwargs)


_RUNNER = None


def _get_runner():
    """Cached jitted shard_map runner over the 8 cores (compile once)."""
    global _RUNNER
    if _RUNNER is not None:
        return _RUNNER
    import jax
    from jax.sharding import Mesh, PartitionSpec
    from jax.experimental.shard_map import shard_map
    from concourse.bass2jax import (
        _bass_exec_p, partition_id_tensor, install_neuronx_cc_hook,
    )

    install_neuronx_cc_hook()
    nc = get_nc()
    partition_name = nc.partition_id_tensor.name if nc.partition_id_tensor else None
    in_names, out_names, out_avals = [], [], []
    for alloc in nc.m.functions[0].allocations:
        if not isinstance(alloc, mybir.MemoryLocationSet):
            continue
        name = alloc.memorylocations[0].name
        if alloc.kind == "ExternalInput":
            if name != partition_name:
                in_names.append(name)
        elif alloc.kind == "ExternalOutput":
            out_names.append(name)
            out_avals.append(jax.core.ShapedArray(
                tuple(alloc.tensor_shape), mybir.dt.np(alloc.dtype)))
    full_in = list(in_names) + list(out_names)
    if partition_name:
        full_in.append(partition_name)

    def _body(*args):
        ops = list(args)
        if partition_name:
            ops.append(partition_id_tensor())
        return tuple(_bass_exec_p.bind(
            *ops,
            out_avals=tuple(out_avals),
            in_names=tuple(full_in),
            out_names=tuple(out_names),
            lowering_input_output_aliases=(),
            sim_require_finite=True,
            sim_require_nnan=True,
            nc=nc,
        ))

    devices = jax.devices()[:NCORES]
    mesh = Mesh(np.asarray(devices), ("core",))
    nin = len(in_names) + len(out_names)
    fn = jax.jit(
        shard_map(_body, mesh=mesh,
                  in_specs=(PartitionSpec("core"),) * nin,
                  out_specs=(PartitionSpec("core"),) * len(out_names),
                  check_rep=False),
        keep_unused=True,
    )
    _RUNNER = (fn, in_names, out_names, out_avals)
    return _RUNNER


def kernel(x, freqs_cos, freqs_sin, wq, wk, wv, wo):
    in_maps = _prep_inputs(x, freqs_cos, freqs_sin, wq, wk, wv, wo)
    try:
        fn, in_names, out_names, out_avals = _get_runner()
        concat_in = [
            np.concatenate([np.asarray(m[name]) for m in in_maps], axis=0)
            for name in in_names
        ]
        concat_zeros = [
            np.zeros((NCORES * a.shape[0], *a.shape[1:]), a.dtype) for a in out_avals
        ]
        outs = fn(*concat_in, *concat_zeros)
        oi = out_names.index("out")
        full = np.asarray(outs[oi]).reshape(NCORES, S, D)
        parts = [full[c] for c in range(NCORES)]
    except Exception:
        res = run(in_maps)
        parts = [res.results[c]["out"] for c in range(NCORES)]
    out = np.stack([parts[2 * b] + parts[2 * b + 1] for b in range(B)], axis=0)
    return out.astype(np.float32)
